# revision 12
# baseline (speedup 1.0000x reference)
"""Multi-head self-attention (RoPE + softmax + out-proj) for Trainium2,
sharded over 8 NeuronCores: data-parallel over batch (4) x tensor-parallel
over heads (2 groups of 8). Each core computes q/k/v projections for its
head group, attention, and a partial output projection; the host sums the
two partials per batch and adds the bias.

Per-core layout highlights:
  - All matmul operands are float32r (rounded fp32): full PE rate at N=512.
  - q/k are produced transposed ([head_dim, n]) directly by projecting
    against Wq / Wkv slices; RoPE's rotate_half is folded into a second,
    host-side column-permuted weight matrix, so RoPE is 3 elementwise DVE
    ops per tile with no partition shuffles.
  - Scores are computed transposed (S^T[m, n]) so softmax's sum over keys m
    becomes a matmul reduction: v is extended with a ones column (M=65
    matmul) whose output row 64 accumulates the softmax denominator.
  - exp runs on the scalar engine straight out of PSUM in 1024-wide
    instructions; normalization uses a DVE reciprocal plus a K=2
    block-diagonal broadcast matmul.
"""

import numpy as np

import concourse.bass as bass
import concourse.mybir as mybir
import concourse.tile as tile

B, N, DIM, H, DH = 4, 2048, 1024, 16, 64
SCALE = DH**-0.5
N_CORES = 8
HG = 8  # heads per core
INNER = HG * DH  # 512, inner dim slice per core
PAIRS = INNER // 128  # 4 head pairs (=128-partition inner chunks)
NB = 4  # n blocks of 512
MB = 16  # m blocks of 128
KD = DIM // 128  # 8 contraction chunks

F32 = mybir.dt.float32
F32R = mybir.dt.float32r
EXP = mybir.ActivationFunctionType.Exp

MAX_WAITS = 1


def _split_excess_waits(nc):
    """This walrus build rejects >1 semaphore wait per instruction; hoist
    excess waits onto nops inserted before the instruction on its engine."""
    import bass_rust

    for f in nc.m.functions:
        for bb in f.blocks:
            il = bb.instructions
            i = 0
            while i < len(il):
                inst = il[i]
                si = inst.sync_info
                if si is not None and si.on_wait and len(si.on_wait) > MAX_WAITS:
                    waits = list(si.on_wait)
                    si.on_wait = waits[:MAX_WAITS]
                    rest = waits[MAX_WAITS:]
                    eng = nc.engines[inst.engine]
                    insert_at = i
                    for j in range(0, len(rest), MAX_WAITS):
                        b = eng.nop(nofuse=True, hint="wait_split")
                        ni = b.ins
                        tail = nc.cur_bb.bb.instructions
                        assert tail[-1] is ni
                        tail.pop()
                        nsi = ni.sync_info
                        if nsi is None:
                            ni.sync_info = bass_rust.SyncInfo(
                                on_wait=rest[j : j + MAX_WAITS], on_update=[]
                            )
                        else:
                            nsi.on_wait = rest[j : j + MAX_WAITS]
                        il.insert(insert_at, ni)
                        insert_at += 1
                        i += 1
                i += 1


class _FixedTileContext(tile.TileContext):
    def __exit__(self, exc_type, exc_val, exc_tb):
        res = super().__exit__(exc_type, exc_val, exc_tb)
        if exc_type is None:
            _split_excess_waits(self.nc)
        return res


def build_kernel():
    nc = bass.Bass()
    xT = nc.dram_tensor("xT", [DIM, N], F32, kind="ExternalInput")
    wq = nc.dram_tensor("wq", [DIM, INNER], F32, kind="ExternalInput")
    wqr = nc.dram_tensor("wqr", [DIM, INNER], F32, kind="ExternalInput")
    wk = nc.dram_tensor("wk", [DIM, INNER], F32, kind="ExternalInput")
    wkr = nc.dram_tensor("wkr", [DIM, INNER], F32, kind="ExternalInput")
    wv = nc.dram_tensor("wv", [DIM, INNER], F32, kind="ExternalInput")
    wo = nc.dram_tensor("wo", [INNER, DIM], F32, kind="ExternalInput")
    cosT = nc.dram_tensor("cosT", [128, N], F32, kind="ExternalInput")
    sinT = nc.dram_tensor("sinT", [128, N], F32, kind="ExternalInput")
    out = nc.dram_tensor("out", [N, DIM], F32, kind="ExternalOutput")

    vs = nc.dram_tensor("vs", [N, INNER], F32R)  # v bounce scratch

    with _FixedTileContext(nc) as tc:
        with (
            tc.tile_pool(name="const", bufs=1) as cpool,
            tc.tile_pool(name="qk", bufs=1) as qkpool,
            tc.tile_pool(name="ps", space=bass.MemorySpace.PSUM, bufs=1) as ps,
            tc.tile_pool(name="io", bufs=1) as iopool,
        ):
            # ---- constants ----
            cos_t = cpool.tile([128, N], F32, tag="cos")
            sin_t = cpool.tile([128, N], F32, tag="sin")
            nc.sync.dma_start(cos_t[:], cosT[:])
            nc.sync.dma_start(sin_t[:], sinT[:])
            ones_f = cpool.tile([128, 64], F32, tag="onesf")
            nc.vector.memset(ones_f[:], 1.0)
            onesr = cpool.tile([128, 64], F32R, tag="onesr")
            nc.vector.tensor_copy(onesr[:], ones_f[:])

            # persistent q^T / k^T (rope applied), [128, pair, n] f32r
            qT = qkpool.tile([128, PAIRS, N], F32R, tag="qT")
            kT = qkpool.tile([128, PAIRS, N], F32R, tag="kT")

            # ---- projection phase ----
            with tc.tile_pool(name="proj", bufs=1) as pj:
                for tgt, w_dram, wr_dram in (
                    (qT, wq, wqr),
                    (kT, wk, wkr),
                    (None, wv, None),
                ):
                    w_t = pj.tile([128, KD, INNER], F32R, tag="w", bufs=2)
                    nc.gpsimd.dma_start(
                        w_t[:], w_dram.rearrange("(c p) i -> p c i", p=128)
                    )
                    if wr_dram is not None:
                        wr_t = pj.tile([128, KD, INNER], F32R, tag="w", bufs=2)
                        nc.gpsimd.dma_start(
                            wr_t[:], wr_dram.rearrange("(c p) i -> p c i", p=128)
                        )
                    for nb in range(NB):
                        x_t = pj.tile([128, KD, 512], F32R, tag="x", bufs=2)
                        nc.gpsimd.dma_start(
                            x_t[:],
                            xT.rearrange("(c p) n -> p c n", p=128)[
                                :, :, nb * 512 : (nb + 1) * 512
                            ],
                        )
                        if wr_dram is not None:
                            # q or k: produce transposed heads + rope
                            for c in range(PAIRS):
                                pq = ps.tile([128, 2, 512], F32, tag="s", bufs=2)
                                for dc in range(KD):
                                    nc.tensor.matmul(
                                        pq[:, 0, :],
                                        w_t[:, dc, c * 128 : (c + 1) * 128],
                                        x_t[:, dc, :],
                                        start=(dc == 0),
                                        stop=(dc == KD - 1),
                                    )
                                for dc in range(KD):
                                    nc.tensor.matmul(
                                        pq[:, 1, :],
                                        wr_t[:, dc, c * 128 : (c + 1) * 128],
                                        x_t[:, dc, :],
                                        start=(dc == 0),
                                        stop=(dc == KD - 1),
                                    )
                                nsl = slice(nb * 512, (nb + 1) * 512)
                                tmp = pj.tile([128, 512], F32, tag="tmp", bufs=3)
                                nc.vector.tensor_mul(
                                    tmp[:], pq[:, 1, :], sin_t[:, nsl]
                                )
                                nc.vector.tensor_mul(
                                    tgt[:, c, nsl], pq[:, 0, :], cos_t[:, nsl]
                                )
                                nc.vector.tensor_add(
                                    tgt[:, c, nsl], tgt[:, c, nsl], tmp[:]
                                )
                        else:
                            # v: natural layout [m, inner], bounce to DRAM
                            for sub in range(4):
                                pv = ps.tile([128, 2, 512], F32, tag="s", bufs=2)
                                for dc in range(KD):
                                    nc.tensor.matmul(
                                        pv[:, 0, :],
                                        x_t[:, dc, sub * 128 : (sub + 1) * 128],
                                        w_t[:, dc, :],
                                        start=(dc == 0),
                                        stop=(dc == KD - 1),
                                    )
                                vstg = pj.tile([128, 512], F32R, tag="vst", bufs=3)
                                nc.vector.tensor_copy(vstg[:], pv[:, 0, :])
                                m0 = nb * 512 + sub * 128
                                nc.sync.dma_start(
                                    vs[m0 : m0 + 128, :], vstg[:]
                                )

            # ---- attention phase ----
            with tc.tile_pool(name="attn", bufs=1) as at:
                otn = []
                for p in range(PAIRS):
                    otn_p = at.tile([128, 4, 512], F32R, tag=f"otn{p}")
                    otn.append(otn_p)
                for p in range(PAIRS):
                    # v for both heads of the pair, extended with ones col
                    vext = []
                    for j in range(2):
                        h = 2 * p + j
                        ve = at.tile([128, MB, 65], F32R, tag="vext", bufs=4)
                        nc.sync.dma_start(
                            ve[:, :, 0:64],
                            vs.rearrange("(mb q) i -> q mb i", q=128)[
                                :, :, h * 64 : (h + 1) * 64
                            ],
                        )
                        for mb in range(MB):
                            nc.vector.tensor_copy(
                                ve[:, mb, 64:65], onesr[:, 0:1]
                            )
                        vext.append(ve)
                    for f in range(2):  # n halves of 1024
                        ot_ab = []
                        for j in range(2):
                            ot = ps.tile([128, 2, 512], F32, tag="ot", bufs=2)
                            ot_ab.append(ot)
                        for mb in range(MB):
                            msl = slice(mb * 128, (mb + 1) * 128)
                            for j in range(2):
                                psl = slice(64 * j, 64 * (j + 1))
                                s_t = ps.tile([128, 2, 512], F32, tag="s", bufs=2)
                                for sub in range(2):
                                    n0 = f * 1024 + sub * 512
                                    nc.tensor.matmul(
                                        s_t[:, sub, :],
                                        kT[psl, p, msl],
                                        qT[psl, p, n0 : n0 + 512],
                                        start=True,
                                        stop=True,
                                    )
                                pt = at.tile([128, 2, 512], F32R, tag="pt", bufs=4)
                                nc.scalar.activation(pt[:], s_t[:], EXP, scale=SCALE)
                                for sub in range(2):
                                    nc.tensor.matmul(
                                        ot_ab[j][0:65, sub, :],
                                        vext[j][:, mb, :],
                                        pt[:, sub, :],
                                        start=(mb == 0),
                                        stop=(mb == MB - 1),
                                    )
                        # softmax denominators -> reciprocal -> broadcast
                        # (head A denom on row 0, head B on row 32; rows
                        # 1-31 hold junk that is never consumed)
                        rin = at.tile([33, 2, 512], F32, tag="rin", bufs=2)
                        nc.vector.tensor_copy(rin[0:1, :, :], ot_ab[0][64:65, :, :])
                        nc.vector.tensor_copy(rin[32:33, :, :], ot_ab[1][64:65, :, :])
                        rec = at.tile([33, 2, 512], F32R, tag="rec", bufs=2)
                        with nc.allow_low_precision(
                            reason="f32r reciprocal for softmax denom"
                        ):
                            # one op covers rows 0..32; rows 1-31 are junk
                            nc.vector.reciprocal(rec[:], rin[:])
                        fsl = slice(f * 2, f * 2 + 2)
                        for j in range(2):
                            bc = ps.tile([128, 2, 512], F32, tag="s", bufs=2)
                            row = 32 * j
                            for sub in range(2):
                                nc.tensor.matmul(
                                    bc[0:64, sub, :],
                                    onesr[row : row + 1, :],
                                    rec[row : row + 1, sub, :],
                                    start=True,
                                    stop=True,
                                )
                            bc_sb = at.tile([64, 2, 512], F32, tag="bcs", bufs=2)
                            nc.vector.tensor_copy(bc_sb[:], bc[0:64, :, :])
                            nc.vector.tensor_mul(
                                otn[p][64 * j : 64 * (j + 1), fsl, :],
                                ot_ab[j][0:64, :, :],
                                bc_sb[:],
                            )

                # ---- output projection (partial; host sums pairs) ----
                wo_t = at.tile([128, PAIRS, DIM], F32R, tag="wo")
                nc.gpsimd.dma_start(
                    wo_t[:], wo.rearrange("(c p) d -> p c d", p=128)
                )
                for nb in range(16):
                    nsl = slice(nb * 128, (nb + 1) * 128)
                    q4, r4 = divmod(nb, 4)
                    for dh in range(2):
                        po = ps.tile([128, 2, 512], F32, tag="s", bufs=2)
                        for c in range(PAIRS):
                            nc.tensor.matmul(
                                po[:, 0, :],
                                otn[c][:, q4, r4 * 128 : (r4 + 1) * 128],
                                wo_t[:, c, dh * 512 : (dh + 1) * 512],
                                start=(c == 0),
                                stop=(c == PAIRS - 1),
                            )
                        ost = iopool.tile([128, 512], F32, tag="ost", bufs=3)
                        nc.any.tensor_copy(ost[:], po[:, 0, :])
                        nc.sync.dma_start(
                            out[nsl, dh * 512 : (dh + 1) * 512], ost[:]
                        )
    return nc


_CACHED = {}


def _get_kernel():
    if "nc" not in _CACHED:
        _CACHED["nc"] = build_kernel()
    return _CACHED["nc"]


def _rot_cols(w):
    # w: [DIM, INNER]; per 64-col head block: out[:, :32] = -w[:, 32:64],
    # out[:, 32:] = w[:, :32]  (so x @ out == rotate_half(x @ w))
    w3 = np.asarray(w, np.float32).reshape(DIM, -1, DH)
    r = np.empty_like(w3)
    r[:, :, : DH // 2] = -w3[:, :, DH // 2 :]
    r[:, :, DH // 2 :] = w3[:, :, : DH // 2]
    return np.ascontiguousarray(r.reshape(DIM, -1))


def kernel(x, rotary_emb_x, Wq, Wkv, Wo, bo):
    from concourse.bass_utils import run_bass_kernel_spmd

    x = np.asarray(x, np.float32)
    rope = np.asarray(rotary_emb_x, np.float32)
    Wq = np.asarray(Wq, np.float32)
    Wkv = np.asarray(Wkv, np.float32)
    Wo = np.asarray(Wo, np.float32)
    bo = np.asarray(bo, np.float32)

    cosT = np.ascontiguousarray(np.cos(rope).T)  # [64, N]
    sinT = np.ascontiguousarray(np.sin(rope).T)
    cosT2 = np.ascontiguousarray(np.concatenate([cosT, cosT], axis=0))
    sinT2 = np.ascontiguousarray(np.concatenate([sinT, sinT], axis=0))

    Wk_full = Wkv[:, : H * DH]
    Wv_full = Wkv[:, H * DH :]

    xTs = [np.ascontiguousarray(x[b].T) for b in range(B)]
    in_maps = []
    for core in range(N_CORES):
        b, hg = divmod(core, 2)
        isl = slice(hg * INNER, (hg + 1) * INNER)
        wq_s = np.ascontiguousarray(Wq[:, isl])
        wk_s = np.ascontiguousarray(Wk_full[:, isl])
        in_maps.append(
            {
                "xT": xTs[b],
                "wq": wq_s,
                "wqr": _rot_cols(wq_s),
                "wk": wk_s,
                "wkr": _rot_cols(wk_s),
                "wv": np.ascontiguousarray(Wv_full[:, isl]),
                "wo": np.ascontiguousarray(Wo[isl, :]),
                "cosT": cosT2,
                "sinT": sinT2,
            }
        )

    nc = _get_kernel()
    _CACHED["in_maps"] = in_maps
    res = run_bass_kernel_spmd(nc, in_maps, list(range(N_CORES)))
    outs = [res.results[i]["out"] for i in range(N_CORES)]
    full = np.stack(
        [outs[2 * b] + outs[2 * b + 1] + bo for b in range(B)], axis=0
    )
    return full
